# revision 4
# baseline (speedup 1.0000x reference)
"""Trainium2 Bass kernel for vq_codebook argmin (nn_GUMSampler).

Per pixel p (4M pixels over 8 cores), picks argmin_v ||z_p - vertex_v|| over
16 vertices in R^15 and the min distance. Pixels shard 8 ways across cores
(contiguous blocks), vertices replicated, no communication.

Math: argmin_v d2 = argmin_v (vv[v] - 2<V_v, z_p>) -- the ||z||^2 term is
pixel-constant, so it never touches the device: the host reconstructs
dmin = sqrt(r_min - C + zz) from the packed device result plus host-side zz.

One fp16 matmul pass per tile: W [128, 128] fp16 block-diagonal,
W[16g+c, 16g+v] = -2V[v,c]; each group's const-1 z-row (16g+15) carries
vv[v]+C split hi/lo across two groups' const rows (exact vv at fp16-pair
precision; C = 34 keeps r positive so f32 bit order = value order).
z ships as fp16 [128, 4096] blocks (half the baseline's bytes); const rows
are packed host-side.

Per iteration (8192 px = 8 groups x 1024 cols):
  PE   : 2 matmuls [128, 512] fp16 + 8 transposes [128,128] f32
  pack : cols [0:XD]  DVE bit-pack (bits(r) & ~15) | (partition & 15)
         cols [XD:]   ACT 2-pass quantize-pack p = i32(r*QS)*16 + v
         (ACT is the only other engine that can pack: Pool has no usable
         tensor ops in this ISA; int bits ride the f32 identity transpose
         as tiny denormals, which CoreSim's permutation transpose preserves)
  DVE  : one tensor_reduce(min) over v on transposed T [p, mb, g, v]
  out  : S [128, 1024] accumulates 16 iters; stored contiguous per epoch
         (final epoch streams 4-iter slabs to shorten the drain)

Software-pipelined with tails trailing heads by LAG_COLS columns so the
in-order PE queue never head-blocks on pack completion.

Host decode: idx = bits & 15; value by column format (bitcast-f32 masked, or
(bits >> 4)/QS); dmin = sqrt(max(val - C + zz, 0)).
"""

import sys

sys.path.insert(0, "/opt/trn_rl_repo")

from contextlib import ExitStack

import numpy as np

import concourse.bacc as bacc
import concourse.tile as tile
from concourse import mybir
from concourse.bass_utils import run_bass_kernel_spmd

F32 = mybir.dt.float32
F16 = mybir.dt.float16
I32 = mybir.dt.int32
COPY = mybir.ActivationFunctionType.Copy
IDENT = mybir.ActivationFunctionType.Identity

K = 16
C = 15
G = 8
N_CORES = 8
LX = LY = 2048
N_TOTAL = LX * LY
N_LOC = N_TOTAL // N_CORES      # 524288
NG = N_LOC // G                 # 65536 pixels per group per core
PX_IT = 8192                    # pixels per iteration (8 groups x 1024)
N_ITERS = N_LOC // PX_IT        # 64
OFF_C = 34.0

AND_MASK = -16
MIN = mybir.AluOpType.min
AND = mybir.AluOpType.bitwise_and
OR = mybir.AluOpType.bitwise_or

XD = 288    # pack columns DVE bit-packs (rest: ACT quantize-pack)
LAG = 2     # software-pipeline depth (tail trails head by LAG iters)
FSPLIT = 0  # first iters split into sub-units
LAG_COLS = 2048
LAG0_COLS = 1024
LAG_RAMP = 4  # iters using LAG0_COLS
LSPLIT = 0  # last iters split into sub-units
QS = 8192.0  # ACT quantize scale: p = int32(round(r*QS))*16 + v

_CACHE = {}


def build_nc(n_iters=N_ITERS):
    assert n_iters % 16 == 0
    n4 = n_iters // 4
    n_ep = n_iters // 16
    nc = bacc.Bacc("TRN2", target_bir_lowering=False, debug=False)

    z_d = nc.dram_tensor("z", [n4, 128, 4096], F16, kind="ExternalInput")
    wq_d = nc.dram_tensor("wq", [128, 128], F16, kind="ExternalInput")
    vvec_d = nc.dram_tensor("vvec", [128, 1], I32, kind="ExternalInput")
    vvf_d = nc.dram_tensor("vvf", [128, 1], F32, kind="ExternalInput")
    ident_d = nc.dram_tensor("ident", [128, 128], F32, kind="ExternalInput")
    out_d = nc.dram_tensor("out", [n_ep, 128, 1024], F32, kind="ExternalOutput")

    with tile.TileContext(nc) as tc, ExitStack() as ctx:
        cpool = ctx.enter_context(tc.tile_pool(name="consts", bufs=1))
        wq_s = cpool.tile([128, 128], F16)
        vvec_s = cpool.tile([128, 1], I32)
        vvf_s = cpool.tile([128, 1], F32)
        ident_s = cpool.tile([128, 128], F32)
        nc.sync.dma_start(wq_s[:], wq_d[:])
        nc.scalar.dma_start(vvec_s[:], vvec_d[:])
        nc.scalar.dma_start(vvf_s[:], vvf_d[:])
        nc.scalar.dma_start(ident_s[:], ident_d[:])

        zpool = ctx.enter_context(tc.tile_pool(name="zt", bufs=1))
        pspool = ctx.enter_context(tc.tile_pool(name="ps", bufs=2, space="PSUM"))
        cppool = ctx.enter_context(tc.tile_pool(name="cp", bufs=2))
        pkpool = ctx.enter_context(tc.tile_pool(name="pk", bufs=3))
        tpool = ctx.enter_context(tc.tile_pool(name="tr", bufs=2, space="PSUM"))
        spool = ctx.enter_context(tc.tile_pool(name="smin", bufs=2))

        # 3 persistent z buffers (4 iters each; const-1 rows shipped in z_d)
        zts = []
        for b in range(3):
            zt = zpool.tile([128, 4096], F16, name=f"z{b}")
            zts.append(zt)

        state = {"S": None, "ps": {}, "pk": {}, "T": {}}

        def emit_head(i, c0, c1):
            zt = zts[(i // 4) % 3]
            if c0 == 0 and i % 4 == 0:
                if i == 0:
                    # chunk the first block so matmul 0 isn't gated on 8KB/
                    # partition of DMA (+900ns sem) before starting
                    for q in range(4):
                        nc.sync.dma_start(
                            zt[:, 1024 * q : 1024 * q + 1024],
                            z_d[0][:, 1024 * q : 1024 * q + 1024],
                        )
                else:
                    nc.sync.dma_start(zt[:], z_d[i // 4])

            col = (i % 4) * 1024
            if c0 == 0:
                state["ps"][i] = pspool.tile([128, 1024], F32, space="PSUM", name="ps")
                state["pk"][i] = pkpool.tile([128, 1024], F32, name="pk")
            ps = state["ps"][i]
            pk = state["pk"][i]
            h0, h1 = c0 // 512, (c1 + 511) // 512
            for h in range(h0, h1):
                nc.tensor.matmul(
                    ps[:, 512 * h : 512 * h + 512],
                    wq_s[:],
                    zt[:, col + 512 * h : col + 512 * h + 512],
                    start=True,
                    stop=True,
                )

            # pack, two formats split by column:
            #  [0:XD]   DVE bit-pack (bits(r)&~15)|v straight from PSUM
            #  [XD:]    ACT quantize-pack p = i32(r*QS)*16 + v (2 passes);
            #           int bits ride as tiny f32 denormals through transpose
            a0, a1 = max(c0, XD), c1
            if a1 > a0:
                qa = cppool.tile([128, a1 - a0], I32, name="qa")
                nc.scalar.activation(
                    qa[:], ps[:, a0:a1], COPY, bias=0.0, scale=QS
                )
                nc.scalar.activation(
                    pk[:, a0:a1].bitcast(I32), qa[:], IDENT,
                    bias=vvf_s[:], scale=16.0,
                )
            d0, d1 = c0, min(c1, XD)
            if d1 > d0:
                nc.vector.tensor_scalar(
                    pk[:, d0:d1].bitcast(I32),
                    ps[:, d0:d1].bitcast(I32),
                    AND_MASK,
                    vvec_s[:],
                    op0=AND,
                    op1=OR,
                )

        def emit_tail(i, c0, c1):
            pk = state["pk"][i]
            if c0 == 0:
                state["T"][i] = tpool.tile([128, 1024], F32, space="PSUM", name="T")
            T = state["T"][i]
            mb0, mb1 = c0 // 128, c1 // 128
            for mb in range(mb0, mb1):
                nc.tensor.transpose(
                    T[:, 128 * mb : 128 * mb + 128],
                    pk[:, 128 * mb : 128 * mb + 128],
                    ident_s[:],
                )

            if i % 16 == 0 and c0 == 0:
                state["S"] = spool.tile([128, 1024], F32, name="S")
            S = state["S"]
            sb = 64 * (i % 16)
            nc.vector.tensor_reduce(
                S[:, sb + 8 * mb0 : sb + 8 * mb1].rearrange(
                    "p (mb g) -> p mb g", mb=mb1 - mb0
                ),
                T[:, c0:c1].rearrange(
                    "p (mb g v) -> p mb g v", mb=mb1 - mb0, v=16
                ),
                axis=mybir.AxisListType.X,
                op=MIN,
            )

            if c1 == 1024:
                del state["ps"][i], state["pk"][i], state["T"][i]
                if n_iters - 16 <= i < n_iters:
                    # final epoch: store 4-iter slabs as they complete to
                    # cut the end-of-kernel drain to the last slab only
                    if i % 4 == 3:
                        sl = slice(64 * ((i % 16) - 3), 64 * (i % 16) + 64)
                        nc.sync.dma_start(out_d[i // 16][:, sl], S[:, sl])
                elif i % 16 == 15:
                    nc.sync.dma_start(out_d[i // 16], S[:])

        # software pipeline over column-units: first/last iters split into
        # 256-col sub-units so fill and drain chains are ~4x shorter; tails
        # trail heads by LAG_COLS columns of emitted work
        units = []
        for i in range(n_iters):
            if i < FSPLIT or i >= n_iters - LSPLIT:
                units += [(i, q * 512, q * 512 + 512) for q in range(2)]
            else:
                units.append((i, 0, 1024))
        pend = []
        hcols = tcols = 0
        for u in units:
            emit_head(*u)
            hcols += u[2] - u[1]
            pend.append(u)
            while pend and hcols - tcols - (pend[0][2] - pend[0][1]) >= (
                LAG0_COLS if pend[0][0] < LAG_RAMP else LAG_COLS
            ):
                v = pend.pop(0)
                emit_tail(*v)
                tcols += v[2] - v[1]
        for v in pend:
            emit_tail(*v)

    nc.compile()
    return nc


def _weights(vertices):
    V = np.asarray(vertices, dtype=np.float32)          # (16, 15)
    vv64 = (V.astype(np.float64) ** 2).sum(1)
    vvC = (vv64 + OFF_C).astype(np.float32)
    vh = vvC.astype(np.float16).astype(np.float32)
    vl = (vvC - vh).astype(np.float32)
    W = np.zeros((128, 128), dtype=np.float32)
    for g in range(G):
        W[16 * g : 16 * g + C, 16 * g : 16 * g + K] = -2.0 * V.T
        W[16 * g + 15, 16 * g : 16 * g + K] = vh
        W[16 * ((g + 1) % G) + 15, 16 * g : 16 * g + K] = vl
    wq = W.astype(np.float16)
    vvec = (np.arange(128, dtype=np.int32) & 15).reshape(128, 1)
    vvf = vvec.astype(np.float32)
    ident = np.eye(128, dtype=np.float32)
    return {"wq": wq, "vvec": vvec, "vvf": vvf, "ident": ident}


def _pack_z(z_fl_core):
    """[15, n_loc] f32 -> [n4, 128, 4096] fp16, row 16g+c = chan c of group g,
    row 16g+15 = 1.0 (const rows for the vv weight rows)."""
    z = np.asarray(z_fl_core, dtype=np.float32)
    n_loc = z.shape[1]
    n4 = n_loc // (G * 4096)
    zb = np.ones((n4, G, 16, 4096), dtype=np.float16)
    zb[:, :, 0:15, :] = (
        z.reshape(C, G, n4, 4096).transpose(2, 1, 0, 3).astype(np.float16)
    )
    return {"z": np.ascontiguousarray(zb.reshape(n4, 128, 4096))}


def make_in_map(z_fl, vertices):
    m = _weights(vertices)
    m.update(_pack_z(z_fl))
    return m


def decode(out_i32, zz, n_loc):
    """out [n_ep, 128, 1024] i32 + host zz -> idx, dmin (core-local order).

    out[e, p, 64*i16 + 8*mb + g] is pixel g*NG + ((e*16+i16)*8 + mb)*128 + p
    """
    n_ep = out_i32.shape[0]
    bits = np.ascontiguousarray(out_i32).view(np.int32)
    b = bits.reshape(n_ep, 128, 16, 8, G)               # [e, p, i16, mb, g]
    b = np.ascontiguousarray(b.transpose(4, 0, 2, 3, 1))  # [g, e, i16, mb, p]
    idx = (b & 15).astype(np.int32)
    # pixel (mb, p) was pk column 128*mb + p: DVE bit-pack format below XD,
    # ACT quantize format at/above it
    dve = (128 * np.arange(8)[:, None] + np.arange(128)[None, :]) < XD
    val = np.where(
        dve[None, None, None, :, :],
        (b & AND_MASK).view(np.float32).astype(np.float64),
        (b >> 4) / QS,
    )
    idx = idx.reshape(n_loc)
    val = val.reshape(n_loc)
    dmin = np.sqrt(np.maximum(val - OFF_C + zz, 0.0)).astype(np.float32)
    return idx, dmin


def kernel(z, vertices):
    z = np.ascontiguousarray(np.asarray(z, dtype=np.float32))
    lx, ly = z.shape[1], z.shape[2]
    n = lx * ly
    z_fl = z.reshape(C, n)
    n_loc = n // N_CORES

    if "nc" not in _CACHE:
        _CACHE["nc"] = build_nc()
    nc = _CACHE["nc"]

    w = _weights(vertices)
    in_maps = []
    for c in range(N_CORES):
        m = dict(w)
        m.update(_pack_z(z_fl[:, c * n_loc : (c + 1) * n_loc]))
        in_maps.append(m)
    res = run_bass_kernel_spmd(nc, in_maps, list(range(N_CORES)))

    zz = np.einsum("ij,ij->j", z_fl, z_fl, dtype=np.float64)
    idx = np.empty(n, dtype=np.int32)
    dmin = np.empty(n, dtype=np.float32)
    for c in range(N_CORES):
        sl = slice(c * n_loc, (c + 1) * n_loc)
        i_c, d_c = decode(res.results[c]["out"], zz[sl], n_loc)
        idx[sl] = i_c
        dmin[sl] = d_c
    return idx.reshape(lx, ly), dmin.reshape(lx, ly)


if __name__ == "__main__":
    print("smoke build only")
    build_nc(16)
    print("ok")


# revision 5
# speedup vs baseline: 1.0133x; 1.0133x over previous
"""Trainium2 Bass kernel for vq_codebook argmin (nn_GUMSampler).

Per pixel p (4M pixels over 8 cores), picks argmin_v ||z_p - vertex_v|| over
16 vertices in R^15 and the min distance. Pixels shard 8 ways across cores
(contiguous blocks), vertices replicated, no communication.

Math: argmin_v d2 = argmin_v (vv[v] - 2<V_v, z_p>) -- the ||z||^2 term is
pixel-constant, so it never touches the device: the host reconstructs
dmin = sqrt(r_min - C + zz) from the packed device result plus host-side zz.

One fp16 matmul pass per tile: W [128, 128] fp16 block-diagonal,
W[16g+c, 16g+v] = -2V[v,c]; each group's const-1 z-row (16g+15) carries
vv[v]+C split hi/lo across two groups' const rows (exact vv at fp16-pair
precision; C = 34 keeps r positive so f32 bit order = value order).
z ships as fp16 [128, 4096] blocks (half the baseline's bytes); const rows
are packed host-side.

Per iteration (8192 px = 8 groups x 1024 cols):
  PE   : 2 matmuls [128, 512] fp16 + 8 transposes [128,128] f32
  pack : cols [0:XD]  DVE bit-pack (bits(r) & ~15) | (partition & 15)
         cols [XD:]   ACT quantize-pack p = i32(r*QS)*16 + v; pass 1
         (PSUM->SBUF quantize) per iteration, pass 2 (SBUF->SBUF *16+v)
         once per iteration PAIR to amortize its access penalty
         (ACT is the only other engine that can pack: Pool has no usable
         tensor ops in this ISA; int bits ride the f32 identity transpose
         as tiny denormals, which CoreSim's permutation transpose preserves)
  DVE  : one tensor_reduce(min) over v on transposed T [p, mb, g, v]
  out  : S [128, 1024] accumulates 16 iters; stored contiguous per epoch
         (final epoch streams 4-iter slabs to shorten the drain)

Software-pipelined with tails trailing heads by LAG_COLS columns so the
in-order PE queue never head-blocks on pack completion.

Host decode: idx = bits & 15; value by column format (bitcast-f32 masked, or
(bits >> 4)/QS); dmin = sqrt(max(val - C + zz, 0)).
"""

import sys

sys.path.insert(0, "/opt/trn_rl_repo")

from contextlib import ExitStack

import numpy as np

import concourse.bacc as bacc
import concourse.tile as tile
from concourse import mybir
from concourse.bass_utils import run_bass_kernel_spmd

F32 = mybir.dt.float32
F16 = mybir.dt.float16
I32 = mybir.dt.int32
COPY = mybir.ActivationFunctionType.Copy
IDENT = mybir.ActivationFunctionType.Identity

K = 16
C = 15
G = 8
N_CORES = 8
LX = LY = 2048
N_TOTAL = LX * LY
N_LOC = N_TOTAL // N_CORES      # 524288
NG = N_LOC // G                 # 65536 pixels per group per core
PX_IT = 8192                    # pixels per iteration (8 groups x 1024)
N_ITERS = N_LOC // PX_IT        # 64
OFF_C = 34.0

AND_MASK = -16
MIN = mybir.AluOpType.min
AND = mybir.AluOpType.bitwise_and
OR = mybir.AluOpType.bitwise_or

XD = 256    # pack columns DVE bit-packs (rest: ACT quantize-pack)
LAG = 2     # software-pipeline depth (tail trails head by LAG iters)
FSPLIT = 0  # first iters split into sub-units
LAG_COLS = 2048
LAG0_COLS = 1024
LAG_RAMP = 4  # iters using LAG0_COLS
LSPLIT = 0  # last iters split into sub-units
QS = 8192.0  # ACT quantize scale: p = int32(round(r*QS))*16 + v

_CACHE = {}


def build_nc(n_iters=N_ITERS):
    assert n_iters % 16 == 0
    n4 = n_iters // 4
    n_ep = n_iters // 16
    nc = bacc.Bacc("TRN2", target_bir_lowering=False, debug=False)

    z_d = nc.dram_tensor("z", [n4, 128, 4096], F16, kind="ExternalInput")
    wq_d = nc.dram_tensor("wq", [128, 128], F16, kind="ExternalInput")
    vvec_d = nc.dram_tensor("vvec", [128, 1], I32, kind="ExternalInput")
    vvf_d = nc.dram_tensor("vvf", [128, 1], F32, kind="ExternalInput")
    ident_d = nc.dram_tensor("ident", [128, 128], F32, kind="ExternalInput")
    out_d = nc.dram_tensor("out", [n_ep, 128, 1024], F32, kind="ExternalOutput")

    with tile.TileContext(nc) as tc, ExitStack() as ctx:
        cpool = ctx.enter_context(tc.tile_pool(name="consts", bufs=1))
        wq_s = cpool.tile([128, 128], F16)
        vvec_s = cpool.tile([128, 1], I32)
        vvf_s = cpool.tile([128, 1], F32)
        ident_s = cpool.tile([128, 128], F32)
        nc.sync.dma_start(wq_s[:], wq_d[:])
        nc.scalar.dma_start(vvec_s[:], vvec_d[:])
        nc.scalar.dma_start(vvf_s[:], vvf_d[:])
        nc.scalar.dma_start(ident_s[:], ident_d[:])

        zpool = ctx.enter_context(tc.tile_pool(name="zt", bufs=1))
        pspool = ctx.enter_context(tc.tile_pool(name="ps", bufs=2, space="PSUM"))
        cppool = ctx.enter_context(tc.tile_pool(name="cp", bufs=2))
        pkpool = ctx.enter_context(tc.tile_pool(name="pk", bufs=3))
        tpool = ctx.enter_context(tc.tile_pool(name="tr", bufs=2, space="PSUM"))
        spool = ctx.enter_context(tc.tile_pool(name="smin", bufs=2))

        # 3 persistent z buffers (4 iters each; const-1 rows shipped in z_d)
        zts = []
        for b in range(3):
            zt = zpool.tile([128, 4096], F16, name=f"z{b}")
            zts.append(zt)

        state = {"S": None, "pk": {}, "qa": {}}

        def emit_head(i):
            zt = zts[(i // 4) % 3]
            if i % 4 == 0:
                if i == 0:
                    # chunk the first block so matmul 0 isn't gated on 8KB/
                    # partition of DMA (+900ns sem) before starting
                    for q in range(4):
                        nc.sync.dma_start(
                            zt[:, 1024 * q : 1024 * q + 1024],
                            z_d[0][:, 1024 * q : 1024 * q + 1024],
                        )
                else:
                    nc.sync.dma_start(zt[:], z_d[i // 4])

            col = (i % 4) * 1024
            ps = pspool.tile([128, 1024], F32, space="PSUM", name="ps")
            for h in (0, 1):
                nc.tensor.matmul(
                    ps[:, 512 * h : 512 * h + 512],
                    wq_s[:],
                    zt[:, col + 512 * h : col + 512 * h + 512],
                    start=True,
                    stop=True,
                )

            # pack, two formats split by column within each iteration:
            #  [0:XD]   DVE bit-pack (bits(r)&~15)|v straight from PSUM
            #  [XD:]    ACT quantize-pack p = i32(r*QS)*16 + v; pass 1
            #           (PSUM->SBUF quantize) runs per iteration, pass 2
            #           (SBUF->SBUF *16+v) runs ONCE PER PAIR over both
            #           iterations' qa halves, amortizing its access penalty
            half = i % 2
            if half == 0:
                state["pk"][i // 2] = pkpool.tile([128, 2048], F32, name="pk")
                state["qa"][i // 2] = cppool.tile(
                    [128, 2 * (1024 - XD)], I32, name="qa"
                )
            pk = state["pk"][i // 2]
            qa = state["qa"][i // 2]
            nya = 1024 - XD
            nc.scalar.activation(
                qa[:, half * nya : half * nya + nya],
                ps[:, XD:1024],
                COPY,
                bias=0.0,
                scale=QS,
            )
            nc.vector.tensor_scalar(
                pk[:, 1024 * half : 1024 * half + XD].bitcast(I32),
                ps[:, 0:XD].bitcast(I32),
                AND_MASK,
                vvec_s[:],
                op0=AND,
                op1=OR,
            )
            if half == 1:
                nc.scalar.activation(
                    pk[:]
                    .bitcast(I32)
                    .rearrange("p (h c) -> p h c", h=2)[:, :, XD:1024],
                    qa[:].rearrange("p (h c) -> p h c", h=2),
                    IDENT,
                    bias=vvf_s[:],
                    scale=16.0,
                )
            return pk

        def emit_tail(i, pk):
            half = i % 2
            T = tpool.tile([128, 1024], F32, space="PSUM", name="T")
            for mb in range(8):
                nc.tensor.transpose(
                    T[:, 128 * mb : 128 * mb + 128],
                    pk[:, 1024 * half + 128 * mb : 1024 * half + 128 * mb + 128],
                    ident_s[:],
                )

            if i % 16 == 0:
                state["S"] = spool.tile([128, 1024], F32, name="S")
            S = state["S"]
            sb = 64 * (i % 16)
            nc.vector.tensor_reduce(
                S[:, sb : sb + 64].rearrange("p (mb g) -> p mb g", mb=8),
                T[:].rearrange("p (mb g v) -> p mb g v", mb=8, v=16),
                axis=mybir.AxisListType.X,
                op=MIN,
            )

            if n_iters - 16 <= i < n_iters:
                # final epoch: store 4-iter slabs as they complete to cut
                # the end-of-kernel drain to the last slab only
                if i % 4 == 3:
                    sl = slice(64 * ((i % 16) - 3), 64 * (i % 16) + 64)
                    nc.sync.dma_start(out_d[i // 16][:, sl], S[:, sl])
            elif i % 16 == 15:
                nc.sync.dma_start(out_d[i // 16], S[:])

        # software pipeline: heads per iteration; tails trail by 2
        # iterations (pass 2 for a pair arrives with the odd head, so even
        # tails see it completed one iteration later)
        pend = []
        for i in range(n_iters):
            pk = emit_head(i)
            pend.append((i, pk))
            if len(pend) > 3:
                j, pkj = pend.pop(0)
                emit_tail(j, pkj)
        for j, pkj in pend:
            emit_tail(j, pkj)

    nc.compile()
    return nc


def _weights(vertices):
    V = np.asarray(vertices, dtype=np.float32)          # (16, 15)
    vv64 = (V.astype(np.float64) ** 2).sum(1)
    vvC = (vv64 + OFF_C).astype(np.float32)
    vh = vvC.astype(np.float16).astype(np.float32)
    vl = (vvC - vh).astype(np.float32)
    W = np.zeros((128, 128), dtype=np.float32)
    for g in range(G):
        W[16 * g : 16 * g + C, 16 * g : 16 * g + K] = -2.0 * V.T
        W[16 * g + 15, 16 * g : 16 * g + K] = vh
        W[16 * ((g + 1) % G) + 15, 16 * g : 16 * g + K] = vl
    wq = W.astype(np.float16)
    vvec = (np.arange(128, dtype=np.int32) & 15).reshape(128, 1)
    vvf = vvec.astype(np.float32)
    ident = np.eye(128, dtype=np.float32)
    return {"wq": wq, "vvec": vvec, "vvf": vvf, "ident": ident}


def _pack_z(z_fl_core):
    """[15, n_loc] f32 -> [n4, 128, 4096] fp16, row 16g+c = chan c of group g,
    row 16g+15 = 1.0 (const rows for the vv weight rows)."""
    z = np.asarray(z_fl_core, dtype=np.float32)
    n_loc = z.shape[1]
    n4 = n_loc // (G * 4096)
    zb = np.ones((n4, G, 16, 4096), dtype=np.float16)
    zb[:, :, 0:15, :] = (
        z.reshape(C, G, n4, 4096).transpose(2, 1, 0, 3).astype(np.float16)
    )
    return {"z": np.ascontiguousarray(zb.reshape(n4, 128, 4096))}


def make_in_map(z_fl, vertices):
    m = _weights(vertices)
    m.update(_pack_z(z_fl))
    return m


def decode(out_i32, zz, n_loc):
    """out [n_ep, 128, 1024] i32 + host zz -> idx, dmin (core-local order).

    out[e, p, 64*i16 + 8*mb + g] is pixel g*NG + ((e*16+i16)*8 + mb)*128 + p
    """
    n_ep = out_i32.shape[0]
    bits = np.ascontiguousarray(out_i32).view(np.int32)
    b = bits.reshape(n_ep, 128, 16, 8, G)               # [e, p, i16, mb, g]
    b = np.ascontiguousarray(b.transpose(4, 0, 2, 3, 1))  # [g, e, i16, mb, p]
    idx = (b & 15).astype(np.int32)
    # pixel (mb, p) was pk column 128*mb + p: DVE bit-pack format below XD,
    # ACT quantize format at/above it
    dve = (128 * np.arange(8)[:, None] + np.arange(128)[None, :]) < XD
    val = np.where(
        dve[None, None, None, :, :],
        (b & AND_MASK).view(np.float32).astype(np.float64),
        (b >> 4) / QS,
    )
    idx = idx.reshape(n_loc)
    val = val.reshape(n_loc)
    dmin = np.sqrt(np.maximum(val - OFF_C + zz, 0.0)).astype(np.float32)
    return idx, dmin


def kernel(z, vertices):
    z = np.ascontiguousarray(np.asarray(z, dtype=np.float32))
    lx, ly = z.shape[1], z.shape[2]
    n = lx * ly
    z_fl = z.reshape(C, n)
    n_loc = n // N_CORES

    if "nc" not in _CACHE:
        _CACHE["nc"] = build_nc()
    nc = _CACHE["nc"]

    w = _weights(vertices)
    in_maps = []
    for c in range(N_CORES):
        m = dict(w)
        m.update(_pack_z(z_fl[:, c * n_loc : (c + 1) * n_loc]))
        in_maps.append(m)
    res = run_bass_kernel_spmd(nc, in_maps, list(range(N_CORES)))

    zz = np.einsum("ij,ij->j", z_fl, z_fl, dtype=np.float64)
    idx = np.empty(n, dtype=np.int32)
    dmin = np.empty(n, dtype=np.float32)
    for c in range(N_CORES):
        sl = slice(c * n_loc, (c + 1) * n_loc)
        i_c, d_c = decode(res.results[c]["out"], zz[sl], n_loc)
        idx[sl] = i_c
        dmin[sl] = d_c
    return idx.reshape(lx, ly), dmin.reshape(lx, ly)


if __name__ == "__main__":
    print("smoke build only")
    build_nc(16)
    print("ok")


# revision 6
# speedup vs baseline: 1.0185x; 1.0051x over previous
"""Trainium2 Bass kernel for vq_codebook argmin (nn_GUMSampler).

Per pixel p (4M pixels over 8 cores), picks argmin_v ||z_p - vertex_v|| over
16 vertices in R^15 and the min distance. Pixels shard 8 ways across cores
(contiguous blocks), vertices replicated, no communication.

Math: argmin_v d2 = argmin_v (vv[v] - 2<V_v, z_p>) -- the ||z||^2 term is
pixel-constant, so it never touches the device: the host reconstructs
dmin = sqrt(r_min - C + zz) from the packed device result plus host-side zz.

One fp16 matmul pass per tile: W [128, 128] fp16 block-diagonal,
W[16g+c, 16g+v] = -2V[v,c]; each group's const-1 z-row (16g+15) carries
vv[v]+C split hi/lo across two groups' const rows (exact vv at fp16-pair
precision; C = 34 keeps r positive so f32 bit order = value order).
z ships as fp16 [128, 4096] blocks (half the baseline's bytes); const rows
are packed host-side.

Per iteration (8192 px = 8 groups x 1024 cols):
  PE   : 2 matmuls [128, 512] fp16 + 8 transposes [128,128] f32
  pack : cols [0:XD]  DVE bit-pack (bits(r) & ~15) | (partition & 15)
         cols [XD:]   ACT quantize-pack p = i32(r*QS)*16 + v; pass 1
         (PSUM->SBUF quantize) per iteration, pass 2 (SBUF->SBUF *16+v)
         once per iteration PAIR to amortize its access penalty
         (ACT is the only other engine that can pack: Pool has no usable
         tensor ops in this ISA; int bits ride the f32 identity transpose
         as tiny denormals, which CoreSim's permutation transpose preserves)
  DVE  : one tensor_reduce(min) over v on transposed T [p, mb, g, v]
  out  : S [128, 1024] accumulates 16 iters; stored contiguous per epoch
         (final epoch streams 4-iter slabs to shorten the drain)

Software-pipelined with tails trailing heads by LAG_COLS columns so the
in-order PE queue never head-blocks on pack completion.

Host decode: idx = bits & 15; value by column format (bitcast-f32 masked, or
(bits >> 4)/QS); dmin = sqrt(max(val - C + zz, 0)).
"""

import sys

sys.path.insert(0, "/opt/trn_rl_repo")

from contextlib import ExitStack

import numpy as np

import concourse.bacc as bacc
import concourse.tile as tile
from concourse import mybir
from concourse.bass_utils import run_bass_kernel_spmd

F32 = mybir.dt.float32
F16 = mybir.dt.float16
I32 = mybir.dt.int32
COPY = mybir.ActivationFunctionType.Copy
IDENT = mybir.ActivationFunctionType.Identity

K = 16
C = 15
G = 8
N_CORES = 8
LX = LY = 2048
N_TOTAL = LX * LY
N_LOC = N_TOTAL // N_CORES      # 524288
NG = N_LOC // G                 # 65536 pixels per group per core
PX_IT = 8192                    # pixels per iteration (8 groups x 1024)
N_ITERS = N_LOC // PX_IT        # 64
OFF_C = 34.0

AND_MASK = -16
MIN = mybir.AluOpType.min
AND = mybir.AluOpType.bitwise_and
OR = mybir.AluOpType.bitwise_or

XD = 246    # pack columns DVE bit-packs (rest: ACT quantize-pack)
LAG = 2     # software-pipeline depth (tail trails head by LAG iters)
FSPLIT = 0  # first iters split into sub-units
LAG_COLS = 2048
LAG0_COLS = 1024
LAG_RAMP = 4  # iters using LAG0_COLS
LSPLIT = 0  # last iters split into sub-units
QS = 8192.0  # ACT quantize scale: p = int32(round(r*QS))*16 + v

_CACHE = {}


def build_nc(n_iters=N_ITERS):
    assert n_iters % 16 == 0
    n4 = n_iters // 4
    n_ep = n_iters // 16
    nc = bacc.Bacc("TRN2", target_bir_lowering=False, debug=False)

    z_d = nc.dram_tensor("z", [n4, 128, 4096], F16, kind="ExternalInput")
    wq_d = nc.dram_tensor("wq", [128, 128], F16, kind="ExternalInput")
    vvec_d = nc.dram_tensor("vvec", [128, 1], I32, kind="ExternalInput")
    vvf_d = nc.dram_tensor("vvf", [128, 1], F32, kind="ExternalInput")
    ident_d = nc.dram_tensor("ident", [128, 128], F32, kind="ExternalInput")
    out_d = nc.dram_tensor("out", [n_ep, 128, 1024], F32, kind="ExternalOutput")

    with tile.TileContext(nc) as tc, ExitStack() as ctx:
        cpool = ctx.enter_context(tc.tile_pool(name="consts", bufs=1))
        wq_s = cpool.tile([128, 128], F16)
        vvec_s = cpool.tile([128, 1], I32)
        vvf_s = cpool.tile([128, 1], F32)
        ident_s = cpool.tile([128, 128], F32)
        nc.sync.dma_start(wq_s[:], wq_d[:])
        nc.scalar.dma_start(vvec_s[:], vvec_d[:])
        nc.scalar.dma_start(vvf_s[:], vvf_d[:])
        nc.scalar.dma_start(ident_s[:], ident_d[:])

        zpool = ctx.enter_context(tc.tile_pool(name="zt", bufs=1))
        pspool = ctx.enter_context(tc.tile_pool(name="ps", bufs=2, space="PSUM"))
        cppool = ctx.enter_context(tc.tile_pool(name="cp", bufs=2))
        pkpool = ctx.enter_context(tc.tile_pool(name="pk", bufs=3))
        tpool = ctx.enter_context(tc.tile_pool(name="tr", bufs=2, space="PSUM"))
        spool = ctx.enter_context(tc.tile_pool(name="smin", bufs=2))

        # 3 persistent z buffers (4 iters each; const-1 rows shipped in z_d)
        zts = []
        for b in range(3):
            zt = zpool.tile([128, 4096], F16, name=f"z{b}")
            zts.append(zt)

        state = {"S": None, "pk": {}, "qa": {}}

        def emit_head(i):
            zt = zts[(i // 4) % 3]
            if i % 4 == 0:
                if i == 0:
                    # chunk the first block so matmul 0 isn't gated on 8KB/
                    # partition of DMA (+900ns sem) before starting
                    for q in range(4):
                        nc.sync.dma_start(
                            zt[:, 1024 * q : 1024 * q + 1024],
                            z_d[0][:, 1024 * q : 1024 * q + 1024],
                        )
                else:
                    nc.sync.dma_start(zt[:], z_d[i // 4])

            col = (i % 4) * 1024
            ps = pspool.tile([128, 1024], F32, space="PSUM", name="ps")
            for h in (0, 1):
                nc.tensor.matmul(
                    ps[:, 512 * h : 512 * h + 512],
                    wq_s[:],
                    zt[:, col + 512 * h : col + 512 * h + 512],
                    start=True,
                    stop=True,
                )

            # pack, two formats split by column within each iteration:
            #  [0:XD]   DVE bit-pack (bits(r)&~15)|v straight from PSUM
            #  [XD:]    ACT quantize-pack p = i32(r*QS)*16 + v; pass 1
            #           (PSUM->SBUF quantize) runs per iteration, pass 2
            #           (SBUF->SBUF *16+v) runs ONCE PER PAIR over both
            #           iterations' qa halves, amortizing its access penalty
            half = i % 2
            if half == 0:
                state["pk"][i // 2] = pkpool.tile([128, 2048], F32, name="pk")
                state["qa"][i // 2] = cppool.tile(
                    [128, 2 * (1024 - XD)], I32, name="qa"
                )
            pk = state["pk"][i // 2]
            qa = state["qa"][i // 2]
            nya = 1024 - XD
            nc.scalar.activation(
                qa[:, half * nya : half * nya + nya],
                ps[:, XD:1024],
                COPY,
                bias=0.0,
                scale=QS,
            )
            nc.vector.tensor_scalar(
                pk[:, 1024 * half : 1024 * half + XD].bitcast(I32),
                ps[:, 0:XD].bitcast(I32),
                AND_MASK,
                vvec_s[:],
                op0=AND,
                op1=OR,
            )
            if half == 1:
                nc.scalar.activation(
                    pk[:]
                    .bitcast(I32)
                    .rearrange("p (h c) -> p h c", h=2)[:, :, XD:1024],
                    qa[:].rearrange("p (h c) -> p h c", h=2),
                    IDENT,
                    bias=vvf_s[:],
                    scale=16.0,
                )
            return pk

        def emit_tail(i, pk):
            half = i % 2
            T = tpool.tile([128, 1024], F32, space="PSUM", name="T")
            for mb in range(8):
                nc.tensor.transpose(
                    T[:, 128 * mb : 128 * mb + 128],
                    pk[:, 1024 * half + 128 * mb : 1024 * half + 128 * mb + 128],
                    ident_s[:],
                )

            if i % 16 == 0:
                state["S"] = spool.tile([128, 1024], F32, name="S")
            S = state["S"]
            sb = 64 * (i % 16)
            nc.vector.tensor_reduce(
                S[:, sb : sb + 64].rearrange("p (mb g) -> p mb g", mb=8),
                T[:].rearrange("p (mb g v) -> p mb g v", mb=8, v=16),
                axis=mybir.AxisListType.X,
                op=MIN,
            )

            if n_iters - 16 <= i < n_iters:
                # final epoch: store 4-iter slabs as they complete to cut
                # the end-of-kernel drain to the last slab only
                if i % 4 == 3:
                    sl = slice(64 * ((i % 16) - 3), 64 * (i % 16) + 64)
                    nc.sync.dma_start(out_d[i // 16][:, sl], S[:, sl])
            elif i % 16 == 15:
                nc.sync.dma_start(out_d[i // 16], S[:])

        # software pipeline: heads per iteration; tails trail by 2
        # iterations (pass 2 for a pair arrives with the odd head, so even
        # tails see it completed one iteration later)
        pend = []
        for i in range(n_iters):
            pk = emit_head(i)
            pend.append((i, pk))
            if len(pend) > 3:
                j, pkj = pend.pop(0)
                emit_tail(j, pkj)
        for j, pkj in pend:
            emit_tail(j, pkj)

    nc.compile()
    return nc


def _weights(vertices):
    V = np.asarray(vertices, dtype=np.float32)          # (16, 15)
    vv64 = (V.astype(np.float64) ** 2).sum(1)
    vvC = (vv64 + OFF_C).astype(np.float32)
    vh = vvC.astype(np.float16).astype(np.float32)
    vl = (vvC - vh).astype(np.float32)
    W = np.zeros((128, 128), dtype=np.float32)
    for g in range(G):
        W[16 * g : 16 * g + C, 16 * g : 16 * g + K] = -2.0 * V.T
        W[16 * g + 15, 16 * g : 16 * g + K] = vh
        W[16 * ((g + 1) % G) + 15, 16 * g : 16 * g + K] = vl
    wq = W.astype(np.float16)
    vvec = (np.arange(128, dtype=np.int32) & 15).reshape(128, 1)
    vvf = vvec.astype(np.float32)
    ident = np.eye(128, dtype=np.float32)
    return {"wq": wq, "vvec": vvec, "vvf": vvf, "ident": ident}


def _pack_z(z_fl_core):
    """[15, n_loc] f32 -> [n4, 128, 4096] fp16, row 16g+c = chan c of group g,
    row 16g+15 = 1.0 (const rows for the vv weight rows)."""
    z = np.asarray(z_fl_core, dtype=np.float32)
    n_loc = z.shape[1]
    n4 = n_loc // (G * 4096)
    zb = np.ones((n4, G, 16, 4096), dtype=np.float16)
    zb[:, :, 0:15, :] = (
        z.reshape(C, G, n4, 4096).transpose(2, 1, 0, 3).astype(np.float16)
    )
    return {"z": np.ascontiguousarray(zb.reshape(n4, 128, 4096))}


def make_in_map(z_fl, vertices):
    m = _weights(vertices)
    m.update(_pack_z(z_fl))
    return m


def decode(out_i32, zz, n_loc):
    """out [n_ep, 128, 1024] i32 + host zz -> idx, dmin (core-local order).

    out[e, p, 64*i16 + 8*mb + g] is pixel g*NG + ((e*16+i16)*8 + mb)*128 + p
    """
    n_ep = out_i32.shape[0]
    bits = np.ascontiguousarray(out_i32).view(np.int32)
    b = bits.reshape(n_ep, 128, 16, 8, G)               # [e, p, i16, mb, g]
    b = np.ascontiguousarray(b.transpose(4, 0, 2, 3, 1))  # [g, e, i16, mb, p]
    idx = (b & 15).astype(np.int32)
    # pixel (mb, p) was pk column 128*mb + p: DVE bit-pack format below XD,
    # ACT quantize format at/above it
    dve = (128 * np.arange(8)[:, None] + np.arange(128)[None, :]) < XD
    val = np.where(
        dve[None, None, None, :, :],
        (b & AND_MASK).view(np.float32).astype(np.float64),
        (b >> 4) / QS,
    )
    idx = idx.reshape(n_loc)
    val = val.reshape(n_loc)
    dmin = np.sqrt(np.maximum(val - OFF_C + zz, 0.0)).astype(np.float32)
    return idx, dmin


def kernel(z, vertices):
    z = np.ascontiguousarray(np.asarray(z, dtype=np.float32))
    lx, ly = z.shape[1], z.shape[2]
    n = lx * ly
    z_fl = z.reshape(C, n)
    n_loc = n // N_CORES

    if "nc" not in _CACHE:
        _CACHE["nc"] = build_nc()
    nc = _CACHE["nc"]

    w = _weights(vertices)
    in_maps = []
    for c in range(N_CORES):
        m = dict(w)
        m.update(_pack_z(z_fl[:, c * n_loc : (c + 1) * n_loc]))
        in_maps.append(m)
    res = run_bass_kernel_spmd(nc, in_maps, list(range(N_CORES)))

    zz = np.einsum("ij,ij->j", z_fl, z_fl, dtype=np.float64)
    idx = np.empty(n, dtype=np.int32)
    dmin = np.empty(n, dtype=np.float32)
    for c in range(N_CORES):
        sl = slice(c * n_loc, (c + 1) * n_loc)
        i_c, d_c = decode(res.results[c]["out"], zz[sl], n_loc)
        idx[sl] = i_c
        dmin[sl] = d_c
    return idx.reshape(lx, ly), dmin.reshape(lx, ly)


if __name__ == "__main__":
    print("smoke build only")
    build_nc(16)
    print("ok")
